# revision 3
# baseline (speedup 1.0000x reference)
"""Causal multi-head attention on 8 Trainium2 NeuronCores.

Problem: B=4, S=2048, D=1024, H=16 heads of hd=64.
Sharding: core c -> batch b = c // 2, head-group g = c % 2 (8 heads each).
Each core computes its batch's attention for its 8 heads plus the partial
output projection (Wo row-slice); the host sums the two partials per batch.

Per-core dataflow (contracted dim always on SBUF partitions; all matmul
inputs bf16, fp32 PSUM accumulation):
  - projections: QT/KT [512, 2048] packed 2 heads per 128-partition tile
    (head 2t in rows 0:64, head 2t+1 in rows 64:128), V [2048, 8*65] with
    a ones column per head (row 64 of the PV output is then the softmax
    denominator).
  - scores computed transposed, ST[k_tile, q]: the two heads of a pair
    run as CONCURRENT K=64 matmuls on disjoint PE row-groups
    (tile_position (0,0) / (64,0) auto-derived from AP base partitions),
    so the 64-dim contraction wastes nothing. Diagonal k-tile i of a
    chunk only computes live cols >= 128*i (scores, exp, and PV all
    trimmed); causal masking multiplies ONE [128,128] triangle tile on
    the single partially-masked 128-col block per diagonal k-tile.
  - exp on ACT straight out of PSUM into bf16 SBUF (no max-subtraction:
    scaled scores are bounded to a few units for this distribution).
  - PV matmuls accumulate ctxT[65, 512] per (head, q-chunk) with
    partial-width accumulation on diagonal tiles; normalize rows 0:64 by
    row 64 via reciprocal + gpsimd partition_broadcast; then Wo.
Emission interleaves projection/Wo matmul groups one-per-j-step inside
the attention loops so the PE queue has filler work during exp waits.
"""

import sys

sys.path.insert(0, "/opt/trn_rl_repo")

from contextlib import ExitStack

import numpy as np

import concourse.tile as tile
from concourse import bacc, mybir
from concourse import bass_utils

F32 = mybir.dt.float32
BF16 = mybir.dt.bfloat16

B, S, D = 4, 2048, 1024
H, HD = 16, 64
NCORES = 8
E = 512          # per-core head span (8 heads * 64)
NHL = 8          # local heads
P = 128
QW = 512         # q-chunk width


def build_program(s=S):
    """Build the single-core Bass program (SPMD across 8 cores)."""
    nqc = s // QW       # q chunks (= projection quarters)
    nd = D // P         # d tiles (contraction for projections)
    net = E // P        # e tiles of QT/KT (head pairs)

    nc = bacc.Bacc("TRN2", target_bir_lowering=False, debug=False)

    xT = nc.dram_tensor("xT", [D, s], BF16, kind="ExternalInput").ap()
    wqT = nc.dram_tensor("wqT", [D, E], BF16, kind="ExternalInput").ap()
    wkT = nc.dram_tensor("wkT", [D, E], BF16, kind="ExternalInput").ap()
    wvT = nc.dram_tensor("wvT", [D, E], BF16, kind="ExternalInput").ap()
    woT = nc.dram_tensor("woT", [E, D], BF16, kind="ExternalInput").ap()
    tri = nc.dram_tensor("tri", [P, P], BF16, kind="ExternalInput").ap()
    onesb = nc.dram_tensor("onesb", [P, 8], BF16, kind="ExternalInput").ap()
    out = nc.dram_tensor("out", [s, D], F32, kind="ExternalOutput").ap()

    with tile.TileContext(nc) as tc, ExitStack() as ctx, \
            nc.allow_low_precision(reason="fp22/bf16 matmul rounding is intended"):
        # --- SBUF pools (all up-front; no address reuse -> no false deps) ---
        pk = ctx.enter_context(tc.tile_pool(name="pk", bufs=1))
        qt = [[pk.tile([P, QW], BF16, tag=f"qt{t}q{q}", name=f"qt{t}q{q}")
               for q in range(nqc)] for t in range(net)]
        kth = [[pk.tile([P, QW], BF16, tag=f"kt{t}q{q}", name=f"kt{t}q{q}")
                for q in range(nqc)] for t in range(net)]
        vt = [pk.tile([P, NHL * 65], BF16, tag=f"v{i}", name=f"v{i}")
              for i in range(nqc * QW // P)]
        trit = pk.tile([P, P], BF16, tag="tri")
        ctxT = [[pk.tile([P, QW], BF16, tag=f"ctx{t}c{q}", name=f"ctxT{t}c{q}")
                 for q in range(nqc)] for t in range(net)]
        wo = [pk.tile([P, D], BF16, tag=f"wo{dt}", name=f"wo{dt}")
              for dt in range(E // P)]
        wq = [pk.tile([P, E], BF16, tag=f"wq{d}", name=f"wq{d}") for d in range(nd)]
        wk = [pk.tile([P, E], BF16, tag=f"wk{d}", name=f"wk{d}") for d in range(nd)]
        wv = [pk.tile([P, E], BF16, tag=f"wv{d}", name=f"wv{d}") for d in range(nd)]
        pt_pool = ctx.enter_context(tc.tile_pool(name="pt", bufs=8))
        inv_pool = ctx.enter_context(tc.tile_pool(name="inv", bufs=2))
        out_pool = ctx.enter_context(tc.tile_pool(name="outp", bufs=4))
        xp = ctx.enter_context(tc.tile_pool(name="xq", bufs=2))

        # --- PSUM pools: st 2x[128,1024] + ctx 2x[65,512] + mm 2x[128,512] ---
        st_ps = ctx.enter_context(tc.tile_pool(name="st_ps", bufs=2, space="PSUM"))
        ctx_ps = ctx.enter_context(tc.tile_pool(name="ctx_ps", bufs=2, space="PSUM"))
        mm_ps = ctx.enter_context(tc.tile_pool(name="mm_ps", bufs=2, space="PSUM"))

        def dma_x(qtr):
            qs = slice(qtr * QW, (qtr + 1) * QW)
            xq = []
            for d in range(nd):
                xtile = xp.tile([P, QW], BF16, tag=f"x{d}", name=f"x{d}_{qtr}")
                nc.sync.dma_start(xtile[:], xT[d * P:(d + 1) * P, qs])
                xq.append(xtile)
            return xq

        def qk_group(qtr, et, xq, w_tiles, dest):
            mm = mm_ps.tile([P, QW], F32, tag="mm", name=f"pj{qtr}_{et}")
            for d in range(nd):
                nc.tensor.matmul(
                    mm[:],
                    w_tiles[d][:, et * P:(et + 1) * P],
                    xq[d][:],
                    start=(d == 0), stop=(d == nd - 1),
                )
            nc.vector.tensor_copy(dest[et][qtr][:], mm[:])

        def v_group(qtr, sti, xq):
            sidx = qtr * (QW // P) + sti
            mm = mm_ps.tile([P, QW], F32, tag="mm", name=f"pv{sidx}")
            for d in range(nd):
                nc.tensor.matmul(
                    mm[:],
                    xq[d][:, sti * P:(sti + 1) * P],
                    wv[d][:],
                    start=(d == 0), stop=(d == nd - 1),
                )
            v_view = vt[sidx][:].rearrange("p (h w) -> p h w", w=65)
            nc.vector.tensor_copy(
                v_view[:, :, 0:64],
                mm[:].rearrange("p (h w) -> p h w", w=64),
            )
            nc.sync.dma_start(
                v_view[:, :, 64:65],
                onesb[:].rearrange("p (a b) -> p a b", b=1),
            )

        def wo_group(c, sti, eo):
            sidx = c * (QW // P) + sti
            ss = slice(sidx * P, (sidx + 1) * P)
            mm = mm_ps.tile([P, QW], F32, tag="mm", name=f"wo{sidx}_{eo}")
            for dt in range(E // P):
                nc.tensor.matmul(
                    mm[:],
                    ctxT[dt][c][:, sti * P:(sti + 1) * P],
                    wo[dt][:, eo * QW:(eo + 1) * QW],
                    start=(dt == 0), stop=(dt == E // P - 1),
                )
            ot = out_pool.tile([P, QW], F32, tag="o", name=f"ot{sidx}_{eo}")
            nc.vector.tensor_copy(ot[:], mm[:])
            nc.sync.dma_start(out[ss, eo * QW:(eo + 1) * QW], ot[:])

        def attn_t(c, t, fillers):
            """Attention for (chunk c, head-pair t); pops one filler per
            j-step so the PE queue has projection/Wo work during exp waits."""
            jmax = 4 * c + 3
            cacc = [ctx_ps.tile([65, QW], F32, tag="ctx",
                                name=f"cacc{c}_{t}_{i}") for i in range(2)]
            prev = None

            def emit_pv(j, pt, off):
                for h in range(2):
                    hh = 2 * t + h
                    nc.tensor.matmul(
                        cacc[h][:, off:],
                        vt[j][:, hh * 65:(hh + 1) * 65],
                        pt[:, h * QW + off:(h + 1) * QW],
                        start=(j == 0),
                        stop=(j == jmax),
                    )

            for j in range(jmax + 1):
                off = max(0, P * (j - 4 * c))
                stp = st_ps.tile([P, 2 * QW], F32, tag="st",
                                 name=f"st{c}_{t}_{j}")
                pt = pt_pool.tile([P, 2 * QW], BF16, tag="pt",
                                  name=f"pt{c}_{t}_{j}")
                kq, kc = j // 4, (j % 4) * P
                for h in range(2):
                    hs = slice(h * 64, (h + 1) * 64)
                    nc.tensor.matmul(
                        stp[:, h * QW + off:(h + 1) * QW],
                        kth[t][kq][hs, kc:kc + P],
                        qt[t][c][hs, off:],
                        start=True, stop=True,
                    )
                if off == 0:
                    nc.scalar.activation(
                        pt[:], stp[:],
                        mybir.ActivationFunctionType.Exp, scale=0.125,
                    )
                else:
                    for h in range(2):
                        nc.scalar.activation(
                            pt[:, h * QW + off:(h + 1) * QW],
                            stp[:, h * QW + off:(h + 1) * QW],
                            mybir.ActivationFunctionType.Exp, scale=0.125,
                        )
                if j >= 4 * c:  # diagonal tile: mask one 128-col block
                    for h in range(2):
                        ms = slice(h * QW + off, h * QW + off + P)
                        nc.vector.tensor_mul(pt[:, ms], pt[:, ms], trit[:])
                if prev is not None:
                    emit_pv(*prev)
                prev = (j, pt, off)
                if fillers:
                    fillers.pop(0)()
            emit_pv(*prev)

            # normalize rows 0..63 by row 64 into ctxT
            for h in range(2):
                hs = slice(h * 64, (h + 1) * 64)
                sums = inv_pool.tile([1, QW], F32, tag="sums",
                                     name=f"sums{c}_{t}_{h}")
                nc.vector.tensor_copy(sums[:], cacc[h][64:65, :])
                rec1 = inv_pool.tile([1, QW], F32, tag="rec1",
                                     name=f"rec1{c}_{t}_{h}")
                scr1 = inv_pool.tile([1, QW], F32, tag="scr1",
                                     name=f"scr1{c}_{t}_{h}")
                nc.vector.reciprocal_approx_accurate(rec1[:], sums[:], scr1[:])
                invb = inv_pool.tile([64, QW], F32, tag="invb",
                                     name=f"invb{c}_{t}_{h}")
                nc.gpsimd.partition_broadcast(invb[:], rec1[:], channels=64)
                nc.vector.tensor_mul(
                    ctxT[t][c][hs, :], cacc[h][0:64, :], invb[:]
                )

        # ---- startup: quarter 0, DMA-ordered so Q matmuls start early ----
        nc.sync.dma_start(trit[:], tri[:])
        xq0 = []
        for d in range(nd):
            xtile = xp.tile([P, QW], BF16, tag=f"x{d}", name=f"x{d}_0")
            nc.sync.dma_start(xtile[:], xT[d * P:(d + 1) * P, 0:QW])
            nc.sync.dma_start(wq[d][:], wqT[d * P:(d + 1) * P, :])
            xq0.append(xtile)
        for et in range(net):
            qk_group(0, et, xq0, wq, qt)
        for d in range(nd):
            nc.sync.dma_start(wk[d][:], wkT[d * P:(d + 1) * P, :])
        for et in range(net):
            qk_group(0, et, xq0, wk, kth)
        for d in range(nd):
            nc.sync.dma_start(wv[d][:], wvT[d * P:(d + 1) * P, :])
        for sti in range(QW // P):
            v_group(0, sti, xq0)
        for dt in range(E // P):
            nc.sync.dma_start(wo[dt][:], woT[dt * P:(dt + 1) * P, :])

        # ---- main: attention chunk c with proj(c+1)/wo(c-1) as fillers ----
        for c in range(nqc):
            fillers = []
            if c + 1 < nqc:
                qtr = c + 1
                xq_next = []

                def make_xdma(qtr=qtr, xq_next=xq_next):
                    def f():
                        xq_next.extend(dma_x(qtr))
                    return f
                fillers.append(make_xdma())
                for et in range(net):
                    fillers.append(lambda qtr=qtr, et=et, xq=xq_next:
                                   qk_group(qtr, et, xq, wq, qt))
                for et in range(net):
                    fillers.append(lambda qtr=qtr, et=et, xq=xq_next:
                                   qk_group(qtr, et, xq, wk, kth))
                for sti in range(QW // P):
                    fillers.append(lambda qtr=qtr, sti=sti, xq=xq_next:
                                   v_group(qtr, sti, xq))
            if c >= 1:
                for sti in range(QW // P):
                    for eo in range(D // QW):
                        fillers.append(lambda c=c - 1, sti=sti, eo=eo:
                                       wo_group(c, sti, eo))
            for t in range(net):
                attn_t(c, t, fillers)
            for f in fillers:
                f()
        for sti in range(QW // P):
            for eo in range(D // QW):
                wo_group(nqc - 1, sti, eo)

    nc.compile()
    return nc


def make_tri():
    """tri[p, qf] = 1.0 iff qf >= p (within-block causal triangle)."""
    qf = np.arange(P)
    p = np.arange(P)[:, None]
    return (qf[None, :] >= p).astype(np.float32)


def shard_inputs(x, Wq, Wk, Wv, Wo):
    import ml_dtypes
    bf = ml_dtypes.bfloat16
    tri = make_tri().astype(bf)
    onesb = np.ones((P, 8), bf)
    in_maps = []
    for core in range(NCORES):
        b, g = core // 2, core % 2
        sl = slice(g * E, (g + 1) * E)
        in_maps.append({
            "xT": np.ascontiguousarray(x[b].T).astype(bf),
            "wqT": np.ascontiguousarray(Wq[sl, :].T).astype(bf),
            "wkT": np.ascontiguousarray(Wk[sl, :].T).astype(bf),
            "wvT": np.ascontiguousarray(Wv[sl, :].T).astype(bf),
            "woT": np.ascontiguousarray(Wo[:, sl].T).astype(bf),
            "tri": tri,
            "onesb": onesb,
        })
    return in_maps


_NC_CACHE = {}


def _get_nc(**kw):
    key = tuple(sorted(kw.items()))
    if key not in _NC_CACHE:
        _NC_CACHE[key] = build_program(**kw)
    return _NC_CACHE[key]


def run(x, Wq, Wk, Wv, Wo, trace=False, **build_kw):
    nc = _get_nc(**build_kw)
    in_maps = shard_inputs(x, Wq, Wk, Wv, Wo)
    res = bass_utils.run_bass_kernel_spmd(
        nc, in_maps, core_ids=list(range(NCORES)), trace=trace,
    )
    outs = [res.results[c]["out"] for c in range(NCORES)]
    full = np.empty((B, S, D), np.float32)
    for b in range(B):
        full[b] = outs[2 * b] + outs[2 * b + 1]
    return full, res


def kernel(x, Wq, Wk, Wv, Wo):
    x = np.asarray(x, np.float32)
    full, _ = run(x, np.asarray(Wq, np.float32), np.asarray(Wk, np.float32),
                  np.asarray(Wv, np.float32), np.asarray(Wo, np.float32))
    return full
